# revision 25
# baseline (speedup 1.0000x reference)
"""Trainium2 Bass kernel for nn_EnsembleSpace (moe_routing).

Reference computation (B=128, E=64, D1=512, D2=2048):
    idx  = top_k(config, 8)                     # [B, E] routing logits
    cfg  = softmax(config * topk_mask)          # full-width softmax
    cfg  = where(cfg < 1e-4, 0, cfg)
    out  = cfg @ kernel.reshape(E, D1*D2)       # [B, D1*D2] -> [B, D1, D2]

Sharding: the big operands are the expert table (256 MB f32, read once)
and the output (512 MB f32, written once).  Sharding the *feature* axis
(D1) over the 8 cores means each core touches 1/8 of both with no
collective at all.

The kernel is purely HBM-bandwidth-bound (all 16 DMA queues ~92% busy in
the f32 version), so the main lever is fp16: the table is converted to
fp16 on the host (16 MB/core read) and the output is written fp16
(32 MB/core) and upconverted on the host.  48 MB/core total vs 96 MB for
f32 — 2x less DMA.  fp16 keeps 10 mantissa bits: measured end-to-end
rel err ~4e-4 vs the 2e-2 gate (PSUM still accumulates in f32).

Each core:
  1. computes the routing weights cfg [128, 64] on-chip in f32 (iterative
     top-8 via 7 max+knockout rounds, exp+sum via one ACT op, eps mask),
  2. transposes cfg to [E, B] via two col-tiled identity matmuls so the
     weights land in BOTH partition halves (rows 0-63 and 64-127),
     downconverting to fp16 in the PSUM->SBUF copy,
  3. streams its table slice as 16 tiles of [128, 4096] fp16 (4 D1-rows
     each, 8 KB/partition DMAs, 12-deep prefetch); each tile runs as 2x8
     row-packed fp16 matmuls (K=64 at array rows 0-63 / 64-127,
     concurrent) paired into 2-bank PSUM tiles so the f32 -> fp16
     PSUM->SBUF casts run as [128, 1024] copies split across DVE and
     ACT, then 2x 1 MB out DMAs.

Measured: ~140 us fast mode / ~154 us when HBM arbitration runs one DMA
queue ~20% slow (environmental, bimodal across runs); f32 baseline was
255-298 us.  48 MB/core at the ~378 GB/s effective per-core HBM rate
bounds the kernel at ~133 us, so the fast mode is ~95% of roofline.

Input DMAs ride the SP HWDGE ring, output DMAs the ACT ring, so the two
streams don't serialize on one descriptor FIFO.
"""

import sys

for _p in ("/opt/trn_rl_repo", "/root/.axon_site/_ro/trn_rl_repo"):
    if _p not in sys.path:
        sys.path.append(_p)

import numpy as np
import concourse.bass as bass
from concourse import tile, masks, bass_utils

mybir = bass.mybir
_f32 = mybir.dt.float32
_f16 = mybir.dt.float16
_X = mybir.AxisListType.X
_alu = mybir.AluOpType

B, E, D1, D2 = 128, 64, 512, 2048
N_CORES = 8
D1_SH = D1 // N_CORES          # 64 D1-rows per core
ROWS_PER_TILE = 4              # D1-rows per input tile
N_TILES = D1_SH // ROWS_PER_TILE   # 16 input tiles of [128, 4096] fp16
TW = 2 * D2                    # tile free width (4096 fp16 = 8 KB/partition)
MM_N = 512                     # one matmul / PSUM bank
TOP_K = 8
SPARSE_EPS = 1e-4

_TRACE = False                 # test.py flips this for profiled runs
_TRACE_KWARGS = {}
LAST_RESULT = None             # BassKernelResults of the last run


def _split_multi_waits(nc):
    """This walrus build rejects >1 sync-wait per instruction.  Tile's
    add_semaphores emits multi-wait instructions (and the kernel-tail drain
    waits on every live semaphore).  Move the extra waits onto same-engine
    nops inserted immediately before the instruction — the engine executes
    serially, so blocking on the nops is equivalent."""
    n_split = 0
    for bb in nc.m.functions[0].blocks:
        out = []
        changed = False
        for inst in bb.instructions:
            si = inst.sync_info
            waits = list(si.on_wait) if (si is not None and si.on_wait) else []
            if len(waits) > 1:
                changed = True
                for w in waits[:-1]:
                    n_split += 1
                    nop = mybir.InstNoOp(name=f"I-waitsplit-{n_split}")
                    nop.engine = inst.engine
                    nop.sync_info = mybir.SyncInfo(on_wait=[w], on_update=[])
                    out.append(nop)
                inst.sync_info = mybir.SyncInfo(
                    on_wait=[waits[-1]], on_update=list(si.on_update or [])
                )
            out.append(inst)
        if changed:
            bb.instructions = out


def _routing_weights(nc, rp, pp, cfg_ap):
    """cfg [B, E] -> cfgT [E, B] fp16 in SBUF (top-8, softmax, eps mask)."""
    cfgin = rp.tile([B, E], _f32, tag="cfgin")
    nc.sync.dma_start(cfgin[:], cfg_ap[:])

    # 8th-largest per row, in exp-space: exp(config) is positive and
    # order-preserving, so "knock out the max" is a 2-op zero-replace
    # (zero can never shadow a remaining value) instead of a 3-op -inf add
    e0 = rp.tile([B, E], _f32, tag="e0")
    nc.scalar.activation(e0[:], cfgin[:], mybir.ActivationFunctionType.Exp)
    t = rp.tile([B, E], _f32, tag="t")
    nc.vector.tensor_copy(t[:], e0[:])
    mk = rp.tile([B, 1], _f32, tag="mk")
    for _ in range(TOP_K - 1):
        nc.vector.reduce_max(mk[:], t[:], axis=_X)
        nc.vector.scalar_tensor_tensor(
            t[:], t[:], mk[:], t[:], op0=_alu.is_lt, op1=_alu.mult
        )
    m8 = rp.tile([B, 1], _f32, tag="m8")
    nc.vector.reduce_max(m8[:], t[:], axis=_X)

    # cfg0 = (exp(config) >= exp(m8)) * config ; softmax ; eps mask
    cfg0 = rp.tile([B, E], _f32, tag="cfg0")
    nc.vector.scalar_tensor_tensor(
        cfg0[:], e0[:], m8[:], cfgin[:], op0=_alu.is_ge, op1=_alu.mult
    )
    ecfg = rp.tile([B, E], _f32, tag="ecfg")
    zs = rp.tile([B, 1], _f32, tag="zs")
    nc.scalar.activation(
        ecfg[:], cfg0[:], mybir.ActivationFunctionType.Exp, accum_out=zs[:]
    )
    rz = rp.tile([B, 1], _f32, tag="rz")
    nc.vector.reciprocal(rz[:], zs[:])
    cfgn = rp.tile([B, E], _f32, tag="cfgn")
    nc.vector.tensor_scalar_mul(cfgn[:], ecfg[:], rz[:])
    cfgf = rp.tile([B, E], _f32, tag="cfgf")
    nc.vector.scalar_tensor_tensor(
        cfgf[:], cfgn[:], SPARSE_EPS, cfgn[:], op0=_alu.is_ge, op1=_alu.mult
    )

    # transpose to [E, B], replicated into both partition halves so the
    # row-packed matmuls can source weights at array rows 0-63 and 64-127;
    # the PSUM->SBUF copy downconverts to fp16 for the fp16 matmuls
    ident = rp.tile([B, B], _f32, tag="ident")
    masks.make_identity(nc, ident[:])
    psT = pp.tile([B, B], _f32, tag="ps")
    nc.tensor.matmul(psT[0:E, :], cfgf[:], ident[:], start=True, stop=True)
    nc.tensor.matmul(psT[E:2 * E, :], cfgf[:], ident[:], start=True, stop=True)
    cfgT2 = rp.tile([B, B], _f16, tag="cfgT2")
    nc.vector.tensor_copy(cfgT2[:], psT[:])
    return cfgT2


def _build():
    nc = bass.Bass(
        "TRN2", target_bir_lowering=False, debug=False, num_devices=N_CORES
    )
    cfg_ap = nc.dram_tensor("config", [B, E], _f32, kind="ExternalInput").ap()
    # input tile t holds D1-rows 4t..4t+3:  partition p = b*64 + e
    # (b = row parity), column c = a*2048 + d2  (a = row pair index),
    # i.e. value = kslice_row(4t + 2a + b)[e, d2]
    ks_ap = nc.dram_tensor(
        "kslice", [N_TILES - 1, 2 * E, TW], _f16, kind="ExternalInput"
    ).ap()
    # the last 4 D1-rows ship as two half-tiles [128, 2048] (2 rows each):
    # the final tile's input lands ~125 us in (queue-FIFO-bound), so its
    # serial compute latency sits directly on the kernel tail — halving
    # the quantum halves that latency
    kst_ap = nc.dram_tensor(
        "kslice_tail", [2, 2 * E, TW // 2], _f16, kind="ExternalInput"
    ).ap()
    # output tile q holds D1-rows 2q (cols 0:2048) and 2q+1 (cols 2048:4096)
    out_ap = nc.dram_tensor(
        "out", [D1_SH // 2, B, TW], _f16, kind="ExternalOutput"
    ).ap()

    with tile.TileContext(nc) as tc:
        # pool depths set the pipeline runway: tile t's first copy waits
        # for the output DMA of tile t-(outp/2) to COMPLETE, and mid-kernel
        # that completion is queue-backlog-bound — a shallow outp pool
        # throttles compute, which starves the tail input DMAs (measured:
        # with outp=8 the last input tiles landed ~70 us late and the DMA
        # queues sagged to ~60-90% busy over the 90-125 us window)
        with tc.tile_pool(name="route", bufs=1) as rp, \
             tc.tile_pool(name="inp", bufs=10) as ip, \
             tc.tile_pool(name="outp", bufs=12) as op_, \
             tc.tile_pool(name="ps", bufs=4, space="PSUM") as pp:
            cfgT2 = _routing_weights(nc, rp, pp, cfg_ap)
            for t in range(N_TILES - 1):
                # all input DMAs ride the sync ring: issuing the priming
                # tiles from ACT delays ACT's exp (the head of the routing
                # chain) by ~2 us of ring writes
                kt = ip.tile([2 * E, TW], _f16, tag="kt")
                nc.sync.dma_start(kt[:], ks_ap[t])
                ot0 = op_.tile([B, TW], _f16, tag="ot")
                ot1 = op_.tile([B, TW], _f16, tag="ot")
                for pi in range(TW // (2 * MM_N)):
                    j0 = 2 * pi * MM_N
                    js0 = slice(j0, j0 + MM_N)
                    js1 = slice(j0 + MM_N, j0 + 2 * MM_N)
                    # kt cols a*2048 + d2, half b  ->  D1-row 4t+2a+b:
                    #   a=0 -> ot0 (rows 4t, 4t+1), a=1 -> ot1 (4t+2, 4t+3)
                    #   psA (b=0, even row) -> cols d2; psB (b=1) -> 2048+d2
                    # two adjacent 512-col matmuls share a 2-bank PSUM tile
                    # so the f32->fp16 cast runs as one [128, 1024] copy
                    ot = ot0 if pi < 2 else ot1
                    oA = j0 % D2
                    osA = slice(oA, oA + 2 * MM_N)
                    osB = slice(oA + D2, oA + D2 + 2 * MM_N)
                    psA = pp.tile([B, 2 * MM_N], _f32, tag="ps")
                    nc.tensor.matmul(
                        psA[:, 0:MM_N], cfgT2[0:E, :], kt[0:E, js0],
                        start=True, stop=True,
                    )
                    nc.tensor.matmul(
                        psA[:, MM_N:], cfgT2[0:E, :], kt[0:E, js1],
                        start=True, stop=True,
                    )
                    psB = pp.tile([B, 2 * MM_N], _f32, tag="ps")
                    nc.tensor.matmul(
                        psB[:, 0:MM_N], cfgT2[E:2 * E, :], kt[E:2 * E, js0],
                        start=True, stop=True,
                    )
                    nc.tensor.matmul(
                        psB[:, MM_N:], cfgT2[E:2 * E, :], kt[E:2 * E, js1],
                        start=True, stop=True,
                    )
                    if pi % 2 == 0:
                        nc.vector.tensor_copy(ot[:, osA], psA[:])
                        nc.scalar.copy(ot[:, osB], psB[:])
                    else:
                        nc.scalar.copy(ot[:, osA], psA[:])
                        nc.vector.tensor_copy(ot[:, osB], psB[:])
                nc.scalar.dma_start(out_ap[2 * t], ot0[:])
                nc.scalar.dma_start(out_ap[2 * t + 1], ot1[:])

            # tail: two half-tiles, rows 60+2h+b -> out tile q = 30+h
            for h in range(2):
                kt = ip.tile([2 * E, TW // 2], _f16, tag="kt")
                nc.sync.dma_start(kt[:], kst_ap[h])
                ot = op_.tile([B, TW], _f16, tag="ot")
                for p in range(2):
                    j0 = p * 2 * MM_N
                    js0 = slice(j0, j0 + MM_N)
                    js1 = slice(j0 + MM_N, j0 + 2 * MM_N)
                    psA = pp.tile([B, 2 * MM_N], _f32, tag="ps")
                    nc.tensor.matmul(
                        psA[:, 0:MM_N], cfgT2[0:E, :], kt[0:E, js0],
                        start=True, stop=True,
                    )
                    nc.tensor.matmul(
                        psA[:, MM_N:], cfgT2[0:E, :], kt[0:E, js1],
                        start=True, stop=True,
                    )
                    psB = pp.tile([B, 2 * MM_N], _f32, tag="ps")
                    nc.tensor.matmul(
                        psB[:, 0:MM_N], cfgT2[E:2 * E, :], kt[E:2 * E, js0],
                        start=True, stop=True,
                    )
                    nc.tensor.matmul(
                        psB[:, MM_N:], cfgT2[E:2 * E, :], kt[E:2 * E, js1],
                        start=True, stop=True,
                    )
                    if p == 0:
                        nc.vector.tensor_copy(ot[:, j0:j0 + 2 * MM_N], psA[:])
                        nc.scalar.copy(ot[:, D2 + j0:D2 + j0 + 2 * MM_N], psB[:])
                    else:
                        nc.scalar.copy(ot[:, j0:j0 + 2 * MM_N], psA[:])
                        nc.vector.tensor_copy(ot[:, D2 + j0:D2 + j0 + 2 * MM_N], psB[:])
                (nc.scalar if h == 0 else nc.sync).dma_start(
                    out_ap[2 * (N_TILES - 1) + h], ot[:]
                )
    _split_multi_waits(nc)
    return nc


_NC_CACHE = None


def _get_nc():
    global _NC_CACHE
    if _NC_CACHE is None:
        _NC_CACHE = _build()
    return _NC_CACHE


def kernel(config, kernel):
    global LAST_RESULT
    config = np.ascontiguousarray(np.asarray(config, dtype=np.float32))
    ktab16 = np.asarray(kernel, dtype=np.float32).reshape(E, D1, D2).astype(
        np.float16
    )

    in_maps = []
    for c in range(N_CORES):
        # [E, D1_SH, D2] -> [D1_SH, E, D2] -> [t, a, b, E, D2] (row=4t+2a+b)
        # -> [t, b, E, a, D2] -> [N_TILES-1, 128, 4096]; the last 4 rows
        # ship as two half-tiles [2(h), 2(b), E, D2] (row = 60+2h+b)
        ksc = ktab16[:, c * D1_SH:(c + 1) * D1_SH, :].transpose(1, 0, 2)
        ksl = (
            ksc[:4 * (N_TILES - 1)]
            .reshape(N_TILES - 1, 2, 2, E, D2)
            .transpose(0, 2, 3, 1, 4)
            .reshape(N_TILES - 1, 2 * E, TW)
        )
        kst = ksc[4 * (N_TILES - 1):].reshape(2, 2 * E, TW // 2)
        in_maps.append(
            {
                "config": config,
                "kslice": np.ascontiguousarray(ksl),
                "kslice_tail": np.ascontiguousarray(kst),
            }
        )

    nc = _get_nc()
    res = None
    for attempt in range(3):
        try:
            res = bass_utils.run_bass_kernel_spmd(
                nc,
                in_maps,
                list(range(N_CORES)),
                trace=_TRACE,
                **_TRACE_KWARGS,
            )
            break
        except Exception:
            # transient NRT device wedge (e.g. NRT_EXEC_UNIT_UNRECOVERABLE)
            # usually clears on retry
            if attempt == 2:
                raise
    LAST_RESULT = res

    out = np.empty((B, D1, D2), dtype=np.float32)
    for c in range(N_CORES):
        # [D1_SH//2, B, 2, D2] (q, b, r, d2) = row 2q+r -> [B, D1_SH, D2]
        o = res.results[c]["out"].reshape(D1_SH // 2, B, 2, D2)
        out[:, c * D1_SH:(c + 1) * D1_SH, :] = (
            o.transpose(1, 0, 2, 3).reshape(B, D1_SH, D2).astype(np.float32)
        )
    return out


# revision 28
# speedup vs baseline: 1.0517x; 1.0517x over previous
"""Trainium2 Bass kernel for nn_EnsembleSpace (moe_routing).

Reference computation (B=128, E=64, D1=512, D2=2048):
    idx  = top_k(config, 8)                     # [B, E] routing logits
    cfg  = softmax(config * topk_mask)          # full-width softmax
    cfg  = where(cfg < 1e-4, 0, cfg)
    out  = cfg @ kernel.reshape(E, D1*D2)       # [B, D1*D2] -> [B, D1, D2]

Sharding: the big operands are the expert table (256 MB f32, read once)
and the output (512 MB f32, written once).  Sharding the *feature* axis
(D1) over the 8 cores means each core touches 1/8 of both with no
collective at all.

The kernel is purely HBM-bandwidth-bound (all 16 DMA queues ~92% busy in
the f32 version), so the main lever is fp16: the table is converted to
fp16 on the host (16 MB/core read) and the output is written fp16
(32 MB/core) and upconverted on the host.  48 MB/core total vs 96 MB for
f32 — 2x less DMA.  fp16 keeps 10 mantissa bits: measured end-to-end
rel err ~4e-4 vs the 2e-2 gate (PSUM still accumulates in f32).

Each core:
  1. computes the routing weights cfg [128, 64] on-chip in f32 (iterative
     top-8 via 7 max+knockout rounds, exp+sum via one ACT op, eps mask),
  2. transposes cfg to [E, B] via two col-tiled identity matmuls so the
     weights land in BOTH partition halves (rows 0-63 and 64-127),
     downconverting to fp16 in the PSUM->SBUF copy,
  3. streams its table slice as 16 tiles of [128, 4096] fp16 (4 D1-rows
     each, 8 KB/partition DMAs, 12-deep prefetch); each tile runs as 2x8
     row-packed fp16 matmuls (K=64 at array rows 0-63 / 64-127,
     concurrent) paired into 2-bank PSUM tiles so the f32 -> fp16
     PSUM->SBUF casts run as [128, 1024] copies split across DVE and
     ACT, then 2x 1 MB out DMAs.

Measured: ~140 us fast mode / ~154 us when HBM arbitration runs one DMA
queue ~20% slow (environmental, bimodal across runs); f32 baseline was
255-298 us.  48 MB/core at the ~378 GB/s effective per-core HBM rate
bounds the kernel at ~133 us, so the fast mode is ~95% of roofline.

Input DMAs ride the SP HWDGE ring, output DMAs the ACT ring, so the two
streams don't serialize on one descriptor FIFO.
"""

import sys

for _p in ("/opt/trn_rl_repo", "/root/.axon_site/_ro/trn_rl_repo"):
    if _p not in sys.path:
        sys.path.append(_p)

import numpy as np
import concourse.bass as bass
from concourse import tile, masks, bass_utils

mybir = bass.mybir
_f32 = mybir.dt.float32
_f16 = mybir.dt.float16
_X = mybir.AxisListType.X
_alu = mybir.AluOpType

B, E, D1, D2 = 128, 64, 512, 2048
N_CORES = 8
D1_SH = D1 // N_CORES          # 64 D1-rows per core
ROWS_PER_TILE = 4              # D1-rows per input tile
N_TILES = D1_SH // ROWS_PER_TILE   # 16 input tiles of [128, 4096] fp16
TW = 2 * D2                    # tile free width (4096 fp16 = 8 KB/partition)
MM_N = 512                     # one matmul / PSUM bank
TOP_K = 8
SPARSE_EPS = 1e-4

_TRACE = False                 # test.py flips this for profiled runs
_TRACE_KWARGS = {}
LAST_RESULT = None             # BassKernelResults of the last run


def _split_multi_waits(nc):
    """This walrus build rejects >1 sync-wait per instruction.  Tile's
    add_semaphores emits multi-wait instructions (and the kernel-tail drain
    waits on every live semaphore).  Move the extra waits onto same-engine
    nops inserted immediately before the instruction — the engine executes
    serially, so blocking on the nops is equivalent."""
    n_split = 0
    for bb in nc.m.functions[0].blocks:
        out = []
        changed = False
        for inst in bb.instructions:
            si = inst.sync_info
            waits = list(si.on_wait) if (si is not None and si.on_wait) else []
            if len(waits) > 1:
                changed = True
                for w in waits[:-1]:
                    n_split += 1
                    nop = mybir.InstNoOp(name=f"I-waitsplit-{n_split}")
                    nop.engine = inst.engine
                    nop.sync_info = mybir.SyncInfo(on_wait=[w], on_update=[])
                    out.append(nop)
                inst.sync_info = mybir.SyncInfo(
                    on_wait=[waits[-1]], on_update=list(si.on_update or [])
                )
            out.append(inst)
        if changed:
            bb.instructions = out


def _routing_weights(nc, rp, pp, cfg_ap):
    """cfg [B, E] -> cfgT [E, B] fp16 in SBUF (top-8, softmax, eps mask)."""
    cfgin = rp.tile([B, E], _f32, tag="cfgin")
    nc.sync.dma_start(cfgin[:], cfg_ap[:])

    # 8th-largest per row, in exp-space: exp(config) is positive and
    # order-preserving, so "knock out the max" is a 2-op zero-replace
    # (zero can never shadow a remaining value) instead of a 3-op -inf add
    e0 = rp.tile([B, E], _f32, tag="e0")
    nc.scalar.activation(e0[:], cfgin[:], mybir.ActivationFunctionType.Exp)
    t = rp.tile([B, E], _f32, tag="t")
    nc.vector.tensor_copy(t[:], e0[:])
    mk = rp.tile([B, 1], _f32, tag="mk")
    for _ in range(TOP_K - 1):
        nc.vector.reduce_max(mk[:], t[:], axis=_X)
        nc.vector.scalar_tensor_tensor(
            t[:], t[:], mk[:], t[:], op0=_alu.is_lt, op1=_alu.mult
        )
    m8 = rp.tile([B, 1], _f32, tag="m8")
    nc.vector.reduce_max(m8[:], t[:], axis=_X)

    # cfg0 = (exp(config) >= exp(m8)) * config ; softmax ; eps mask
    cfg0 = rp.tile([B, E], _f32, tag="cfg0")
    nc.vector.scalar_tensor_tensor(
        cfg0[:], e0[:], m8[:], cfgin[:], op0=_alu.is_ge, op1=_alu.mult
    )
    ecfg = rp.tile([B, E], _f32, tag="ecfg")
    zs = rp.tile([B, 1], _f32, tag="zs")
    nc.scalar.activation(
        ecfg[:], cfg0[:], mybir.ActivationFunctionType.Exp, accum_out=zs[:]
    )
    rz = rp.tile([B, 1], _f32, tag="rz")
    nc.vector.reciprocal(rz[:], zs[:])
    cfgn = rp.tile([B, E], _f32, tag="cfgn")
    nc.vector.tensor_scalar_mul(cfgn[:], ecfg[:], rz[:])
    cfgf = rp.tile([B, E], _f32, tag="cfgf")
    nc.vector.scalar_tensor_tensor(
        cfgf[:], cfgn[:], SPARSE_EPS, cfgn[:], op0=_alu.is_ge, op1=_alu.mult
    )

    # transpose to [E, B], replicated into both partition halves so the
    # row-packed matmuls can source weights at array rows 0-63 and 64-127;
    # the PSUM->SBUF copy downconverts to fp16 for the fp16 matmuls
    ident = rp.tile([B, B], _f32, tag="ident")
    masks.make_identity(nc, ident[:])
    psT = pp.tile([B, B], _f32, tag="ps")
    nc.tensor.matmul(psT[0:E, :], cfgf[:], ident[:], start=True, stop=True)
    nc.tensor.matmul(psT[E:2 * E, :], cfgf[:], ident[:], start=True, stop=True)
    cfgT2 = rp.tile([B, B], _f16, tag="cfgT2")
    nc.vector.tensor_copy(cfgT2[:], psT[:])
    return cfgT2


def _build():
    nc = bass.Bass(
        "TRN2", target_bir_lowering=False, debug=False, num_devices=N_CORES
    )
    cfg_ap = nc.dram_tensor("config", [B, E], _f32, kind="ExternalInput").ap()
    # input tile t holds D1-rows 4t..4t+3:  partition p = b*64 + e
    # (b = row parity), column c = a*2048 + d2  (a = row pair index),
    # i.e. value = kslice_row(4t + 2a + b)[e, d2]
    ks_ap = nc.dram_tensor(
        "kslice", [N_TILES - 1, 2 * E, TW], _f16, kind="ExternalInput"
    ).ap()
    # the last 4 D1-rows ship as two half-tiles [128, 2048] (2 rows each):
    # the final tile's input lands ~125 us in (queue-FIFO-bound), so its
    # serial compute latency sits directly on the kernel tail — halving
    # the quantum halves that latency
    kst_ap = nc.dram_tensor(
        "kslice_tail", [2, 2 * E, TW // 2], _f16, kind="ExternalInput"
    ).ap()
    # output tile q holds D1-rows 2q (cols 0:2048) and 2q+1 (cols 2048:4096)
    out_ap = nc.dram_tensor(
        "out", [D1_SH // 2, B, TW], _f16, kind="ExternalOutput"
    ).ap()

    with tile.TileContext(nc) as tc:
        # pool depths set the pipeline runway: tile t's first copy waits
        # for the output DMA of tile t-(outp/2) to COMPLETE, and mid-kernel
        # that completion is queue-backlog-bound — a shallow outp pool
        # throttles compute, which starves the tail input DMAs (measured:
        # with outp=8 the last input tiles landed ~70 us late and the DMA
        # queues sagged to ~60-90% busy over the 90-125 us window)
        with tc.tile_pool(name="route", bufs=1) as rp, \
             tc.tile_pool(name="inp", bufs=10) as ip, \
             tc.tile_pool(name="outp", bufs=12) as op_, \
             tc.tile_pool(name="ps", bufs=4, space="PSUM") as pp:
            cfgT2 = _routing_weights(nc, rp, pp, cfg_ap)
            for t in range(N_TILES - 1):
                # all input DMAs ride the sync ring: issuing the priming
                # tiles from ACT delays ACT's exp (the head of the routing
                # chain) by ~2 us of ring writes
                kt = ip.tile([2 * E, TW], _f16, tag="kt")
                nc.sync.dma_start(kt[:], ks_ap[t])
                ot0 = op_.tile([B, TW], _f16, tag="ot")
                ot1 = op_.tile([B, TW], _f16, tag="ot")
                for pi in range(TW // (2 * MM_N)):
                    j0 = 2 * pi * MM_N
                    js0 = slice(j0, j0 + MM_N)
                    js1 = slice(j0 + MM_N, j0 + 2 * MM_N)
                    # kt cols a*2048 + d2, half b  ->  D1-row 4t+2a+b:
                    #   a=0 -> ot0 (rows 4t, 4t+1), a=1 -> ot1 (4t+2, 4t+3)
                    #   psA (b=0, even row) -> cols d2; psB (b=1) -> 2048+d2
                    # two adjacent 512-col matmuls share a 2-bank PSUM tile
                    # so the f32->fp16 cast runs as one [128, 1024] copy
                    ot = ot0 if pi < 2 else ot1
                    oA = j0 % D2
                    osA = slice(oA, oA + 2 * MM_N)
                    osB = slice(oA + D2, oA + D2 + 2 * MM_N)
                    psA = pp.tile([B, 2 * MM_N], _f32, tag="ps")
                    nc.tensor.matmul(
                        psA[:, 0:MM_N], cfgT2[0:E, :], kt[0:E, js0],
                        start=True, stop=True,
                    )
                    nc.tensor.matmul(
                        psA[:, MM_N:], cfgT2[0:E, :], kt[0:E, js1],
                        start=True, stop=True,
                    )
                    psB = pp.tile([B, 2 * MM_N], _f32, tag="ps")
                    nc.tensor.matmul(
                        psB[:, 0:MM_N], cfgT2[E:2 * E, :], kt[E:2 * E, js0],
                        start=True, stop=True,
                    )
                    nc.tensor.matmul(
                        psB[:, MM_N:], cfgT2[E:2 * E, :], kt[E:2 * E, js1],
                        start=True, stop=True,
                    )
                    if pi % 2 == 0:
                        nc.vector.tensor_copy(ot[:, osA], psA[:])
                        nc.scalar.copy(ot[:, osB], psB[:])
                    else:
                        nc.scalar.copy(ot[:, osA], psA[:])
                        nc.vector.tensor_copy(ot[:, osB], psB[:])
                nc.scalar.dma_start(out_ap[2 * t], ot0[:])
                nc.scalar.dma_start(out_ap[2 * t + 1], ot1[:])

            # tail: two half-tiles, rows 60+2h+b -> out tile q = 30+h
            for h in range(2):
                kt = ip.tile([2 * E, TW // 2], _f16, tag="kt")
                nc.sync.dma_start(kt[:], kst_ap[h])
                ot = op_.tile([B, TW], _f16, tag="ot")
                for p in range(2):
                    j0 = p * 2 * MM_N
                    js0 = slice(j0, j0 + MM_N)
                    js1 = slice(j0 + MM_N, j0 + 2 * MM_N)
                    psA = pp.tile([B, 2 * MM_N], _f32, tag="ps")
                    nc.tensor.matmul(
                        psA[:, 0:MM_N], cfgT2[0:E, :], kt[0:E, js0],
                        start=True, stop=True,
                    )
                    nc.tensor.matmul(
                        psA[:, MM_N:], cfgT2[0:E, :], kt[0:E, js1],
                        start=True, stop=True,
                    )
                    psB = pp.tile([B, 2 * MM_N], _f32, tag="ps")
                    nc.tensor.matmul(
                        psB[:, 0:MM_N], cfgT2[E:2 * E, :], kt[E:2 * E, js0],
                        start=True, stop=True,
                    )
                    nc.tensor.matmul(
                        psB[:, MM_N:], cfgT2[E:2 * E, :], kt[E:2 * E, js1],
                        start=True, stop=True,
                    )
                    if p == 0:
                        nc.vector.tensor_copy(ot[:, j0:j0 + 2 * MM_N], psA[:])
                        nc.scalar.copy(ot[:, D2 + j0:D2 + j0 + 2 * MM_N], psB[:])
                    else:
                        nc.scalar.copy(ot[:, j0:j0 + 2 * MM_N], psA[:])
                        nc.vector.tensor_copy(ot[:, D2 + j0:D2 + j0 + 2 * MM_N], psB[:])
                (nc.scalar if h == 0 else nc.sync).dma_start(
                    out_ap[2 * (N_TILES - 1) + h], ot[:]
                )
    _split_multi_waits(nc)
    return nc


_NC_CACHE = None


def _get_nc():
    global _NC_CACHE
    if _NC_CACHE is None:
        _NC_CACHE = _build()
    return _NC_CACHE


def kernel(config, kernel):
    global LAST_RESULT
    config = np.ascontiguousarray(np.asarray(config, dtype=np.float32))
    ktab16 = np.asarray(kernel, dtype=np.float32).reshape(E, D1, D2).astype(
        np.float16
    )

    in_maps = []
    for c in range(N_CORES):
        # [E, D1_SH, D2] -> [D1_SH, E, D2] -> [t, a, b, E, D2] (row=4t+2a+b)
        # -> [t, b, E, a, D2] -> [N_TILES, 128, 4096]
        ksc = ktab16[:, c * D1_SH:(c + 1) * D1_SH, :].transpose(1, 0, 2)
        ksl = (
            ksc[:4 * (N_TILES - 1)]
            .reshape(N_TILES - 1, 2, 2, E, D2)
            .transpose(0, 2, 3, 1, 4)
            .reshape(N_TILES - 1, 2 * E, TW)
        )
        kst = ksc[4 * (N_TILES - 1):].reshape(2, 2 * E, TW // 2)
        in_maps.append(
            {
                "config": config,
                "kslice": np.ascontiguousarray(ksl),
                "kslice_tail": np.ascontiguousarray(kst),
            }
        )

    nc = _get_nc()
    res = None
    for attempt in range(3):
        try:
            res = bass_utils.run_bass_kernel_spmd(
                nc,
                in_maps,
                list(range(N_CORES)),
                trace=_TRACE,
                **_TRACE_KWARGS,
            )
            break
        except Exception:
            # transient NRT device wedge (e.g. NRT_EXEC_UNIT_UNRECOVERABLE)
            # usually clears on retry
            if attempt == 2:
                raise
    LAST_RESULT = res

    out = np.empty((B, D1, D2), dtype=np.float32)
    for c in range(N_CORES):
        # [D1_SH//2, B, 2, D2] (q, b, r, d2) = row 2q+r -> [B, D1_SH, D2]
        o = res.results[c]["out"].reshape(D1_SH // 2, B, 2, D2)
        out[:, c * D1_SH:(c + 1) * D1_SH, :] = (
            o.transpose(1, 0, 2, 3).reshape(B, D1_SH, D2).astype(np.float32)
        )
    return out
